# revision 38
# baseline (speedup 1.0000x reference)
"""Trainium2 Bass kernel for nn_CascadedAttention (B=64, T=512, D=1024, V=28).

Math notes (why this is NOT a 512-step sequential scan on device):

  reference computes, per step t with carry y_prev (y_{-1} = 0):
    scores = softmax(tanh(...) @ Va, axis=-1)     # softmax over a SIZE-1 axis
                                                  # -> exactly 1.0 everywhere
    c      = einsum('btd,bt->bd', x, scores)      # -> x.sum(axis=1), step-invariant
    idx    = int32(y_prev)                        # y_prev in (0,1] -> idx in {0,1};
                                                  # idx==1 iff y_prev == 1.0 (fp32-saturated sigmoid)
    WoE    = emb_table[idx] @ Wo                  # -> w0 + (w1-w0)*idx elementwise
    y      = sigmoid(WoE + h_prev @ Uo + c @ Co)  # h_prev = x[:, t-1] (0 at t=0)

  So with a_t[b,v] = w0 + (c@Co)[b,v] + (x[:,t-1]@Uo)[b,v]  (the t=0 column has no
  G term), delta = w1 - w0, and the binary state b_t = 1[a_t + delta*b_{t-1} >= theta]
  (theta = fp32 sigmoid saturation threshold), the outputs are
      y_t = sigmoid(a_t + delta * b_{t-1}).
  b_t follows p0_t + (p1_t - p0_t)*b_{t-1} with p0 = 1[a>=theta], p1 = 1[a>=theta-delta],
  which maps exactly onto the DVE tensor_tensor_scan primitive
  (state = data0*state + data1), i.e. ONE instruction per batch.

  Wa, Ua, Va are mathematically dead (they only feed the all-ones softmax).

Sharding: data-parallel over batch, 8 batches per core. x is pre-transposed on
host to [BS, D, T] so x loads are fully contiguous.

Toolchain constraint that shaped the structure: this walrus build allows ONE
sync wait per instruction (and ~11 on the tail drain). Hence: few large DMAs
(one per DMAHW bookkeeping lane, so no lane-predecessor waits), warm-up
consumers per engine for the constants, single-writer-engine tiles for the scan
stage, and a single lane-first output store whose only wait is the sigmoid.
"""

import numpy as np

import concourse.bass as bass
import concourse.mybir as mybir
import concourse.tile as _tile_mod
from concourse.tile import TileContext
from concourse.vector_clock import ScopedClock
from concourse.bass_utils import run_bass_kernel_spmd


def _split_drain_and_barrier(self, tick_clock, wait_clock):
    """Replacement for TileContext._drain_and_barrier: the nix walrus build
    rejects any instruction with more than one sync wait ("Too many sync wait
    commands", CTRL_NO setupSyncWait), so the tail drain's N-sem wait list is
    split into a chain of single-wait drains on SP. Semantically identical:
    SP executes them in order, so all procs are quiesced before sem teardown.
    """
    nc = self.nc
    drain_inst = nc.sync.drain()
    wait_clock.add_sem_waits(
        drain_inst.ins, ScopedClock({None: tick_clock.global_clock})
    )
    si = drain_inst.ins.sync_info
    waits = list(si.on_wait) if si is not None and si.on_wait else []
    upds = list(si.on_update) if si is not None and si.on_update else []
    if len(waits) > 1:
        drain_inst.ins.sync_info = mybir.SyncInfo(on_wait=[waits[0]], on_update=[])
        for i, w in enumerate(waits[1:]):
            d2 = nc.sync.drain()
            last = i == len(waits) - 2
            d2.ins.sync_info = mybir.SyncInfo(
                on_wait=[w], on_update=upds if last else []
            )

    nc.all_engine_barrier()
    assert self.sems is not None
    popped = nc._tile_sem_poison_stack.pop()
    assert popped is self._sem_poison
    nc.clear_and_free_semaphores(list(self.sems.allocated().values()))
    nc.all_engine_barrier()


_tile_mod.TileContext._drain_and_barrier = _split_drain_and_barrier

B, T, D, V = 64, 512, 1024, 28
N_CORES = 8
BS = B // N_CORES          # batches per core
KC = D // 128              # contraction chunks
F32 = mybir.dt.float32
# smallest fp32 x with 1/(1+exp(-x)) == 1.0 (24*ln2). Any value in [16, 19]
# yields indistinguishable outputs (see derivation above: a theta mismatch only
# flips idx where the NEXT sigmoid is saturated, shifting y by < 1e-6).
THETA = 16.635532333438687

CO = 32        # partition offset of the Co block in psum (32-aligned)
CW = 64        # per-chunk weight columns: 0:28 Uo, 28:32 pad, 32:60 Co, 60:64 pad
NCONST = KC * CW + 4   # packed consts row: 8 weight chunks + [w0, delta, theta, theta-delta]

_NC_CACHE: dict = {}


def _build_nc() -> bass.Bass:
    nc = bass.Bass()
    xt = nc.declare_dram_parameter("xt", [BS, D, T], F32, isOutput=False)
    consts = nc.declare_dram_parameter("consts", [128, NCONST], F32, isOutput=False)
    out = nc.declare_dram_parameter("out", [V, BS * T], F32, isOutput=True)

    with TileContext(nc) as tc:
        with (
            tc.tile_pool(name="consts_p", bufs=1) as cpool,
            tc.tile_pool(name="xin", bufs=1) as xpool,
            tc.tile_pool(name="mid", bufs=2) as mpool,
            tc.tile_pool(name="scan", bufs=2) as spool,
            tc.tile_pool(name="psum", bufs=BS, space="PSUM") as ppool,
        ):
            # one DMA for all constants: weight chunk k lives at columns
            # [k*CW, k*CW+60), wd scalars at the tail
            cb = cpool.tile([128, NCONST], F32)
            nc.sync.dma_start(out=cb[:], in_=consts[:])
            WD = KC * CW  # column where [w0, delta, theta, theta-delta] starts
            # DVE warm-up consumption so later DVE users carry no DMA wait
            junk = cpool.tile([1, 4], F32)
            nc.vector.tensor_copy(junk[:], cb[0:1, WD:WD + 4])

            # all 8 batches' z values side by side; single writer engine (DVE),
            # per-batch slices -> no slot recycling, minimal cross-engine waits
            z_all = cpool.tile([V, BS * T], F32)
            y_all = cpool.tile([V, BS * T], F32)

            ps_tiles = [
                ppool.tile([CO + V, T], F32, tag="ps", name=f"ps{i}")
                for i in range(BS)
            ]
            # PE warm-up matmul consuming the consts DMA so no later matmul
            # needs more than one wait
            nc.tensor.matmul(
                ps_tiles[0][0:1, 0:1], cb[:, 0:1], cb[:, 0:1],
                start=True, stop=True,
            )

            # x arrives in 4 large DMAs (2 batches each). 6 DMAs total in the
            # kernel -> every DMA is first on its bookkeeping lane (no
            # lane-predecessor waits) and the tail drain stays under its cap.
            xp_tiles = []
            for i in range(BS // 2):
                xp = xpool.tile([128, 2, KC, T], F32, tag=f"xp{i}", name=f"xp{i}")
                nc.sync.dma_start(
                    out=xp[:],
                    in_=xt[2 * i:2 * i + 2].rearrange("b (k p) t -> p b k t", p=128),
                )
                xp_tiles.append(xp)

            for b in range(BS):
                # ps[0:28]  = G.T  = Uo.T @ x[b].T      [V, T]
                # ps[32:60] = CC.T = Co.T @ x[b].T      [V, T]
                ps = ps_tiles[b]
                xp = xp_tiles[b // 2]
                for k in range(KC):
                    nc.tensor.matmul(
                        ps[:], cb[:, k * CW:k * CW + CO + V], xp[:, b % 2, k, :],
                        start=(k == 0), stop=(k == KC - 1),
                    )

                # bias = w0 + sum_t CC.T[:, t]  ( = w0 + (c @ Co)[b] )
                bias = mpool.tile([CO + V, 1], F32, tag="bias")
                nc.vector.tensor_reduce(
                    out=bias[CO:CO + V, :], in_=ps[CO:CO + V, :],
                    axis=mybir.AxisListType.X, op=mybir.AluOpType.add,
                )
                nc.vector.tensor_scalar_add(
                    bias[CO:CO + V, :], bias[CO:CO + V, :], cb[CO:CO + V, WD:WD + 1]
                )

                # a[:, t] = G.T[:, t-1] + bias  (t>=1);  a[:, 0] = bias
                # (on DVE: its clock already covers PE's psum-stop via the
                # reduce above, so this adds no new cross-engine wait)
                a = mpool.tile([V, T], F32, tag="a")
                nc.vector.tensor_scalar(
                    out=a[:, 1:T], in0=ps[0:V, 0:T - 1],
                    scalar1=bias[CO:CO + V, 0:1], scalar2=None,
                    op0=mybir.AluOpType.add,
                )
                nc.vector.tensor_copy(a[:, 0:1], bias[CO:CO + V, :])

                # binary saturation state via the native prefix scan
                p0 = spool.tile([V, T], F32, tag="p0")
                d01 = spool.tile([V, T], F32, tag="d01")
                bt = spool.tile([V, T], F32, tag="bt")
                nc.vector.tensor_scalar(
                    out=p0[:], in0=a[:], scalar1=cb[0:V, WD + 2:WD + 3], scalar2=None,
                    op0=mybir.AluOpType.is_ge,
                )
                nc.vector.tensor_scalar(
                    out=d01[:], in0=a[:], scalar1=cb[0:V, WD + 3:WD + 4], scalar2=None,
                    op0=mybir.AluOpType.is_ge,
                )
                nc.vector.tensor_sub(d01[:], d01[:], p0[:])
                # state_t = d01_t * state_{t-1} + p0_t   (exact on {0,1})
                nc.vector.tensor_tensor_scan(
                    out=bt[:], data0=d01[:], data1=p0[:], initial=0.0,
                    op0=mybir.AluOpType.mult, op1=mybir.AluOpType.add,
                )

                # z_t = a_t + delta * b_{t-1}
                zb = b * T
                nc.vector.scalar_tensor_tensor(
                    out=z_all[:, zb + 1:zb + T], in0=bt[:, 0:T - 1],
                    scalar=cb[0:V, WD + 1:WD + 2], in1=a[:, 1:T],
                    op0=mybir.AluOpType.mult, op1=mybir.AluOpType.add,
                )
                nc.vector.tensor_copy(z_all[:, zb:zb + 1], a[:, 0:1])

            # y = sigmoid(z) in two chunks (ACT/DVE overlap), then ONE store,
            # first on its DMA lane: its only wait is the final sigmoid
            H = BS * T // 2
            for h in range(2):
                nc.scalar.activation(
                    out=y_all[:, h * H:(h + 1) * H], in_=z_all[:, h * H:(h + 1) * H],
                    func=mybir.ActivationFunctionType.Sigmoid,
                )
            nc.sync.dma_start(out=out[:], in_=y_all[:])

    return nc


def _host_smalls(Wo, Uo, Co, emb_table):
    w0 = np.float32(emb_table[0].astype(np.float32) @ Wo[:, 0].astype(np.float32))
    w1 = np.float32(emb_table[1].astype(np.float32) @ Wo[:, 0].astype(np.float32))
    delta = np.float32(w1 - w0)
    theta = np.float32(THETA)
    uoco = np.zeros((D, CW), np.float32)
    uoco[:, 0:V] = Uo
    uoco[:, CO:CO + V] = Co
    # packed consts: row p holds the 8 weight chunks then the 4 scalars
    consts = np.zeros((128, NCONST), np.float32)
    consts[:, 0:KC * CW] = (
        uoco.reshape(KC, 128, CW).transpose(1, 0, 2).reshape(128, KC * CW)
    )
    consts[:, KC * CW:] = np.array(
        [w0, delta, theta, np.float32(theta - delta)], np.float32
    )
    return np.ascontiguousarray(consts)


def _in_maps(x, Wo, Uo, Co, emb_table):
    x = np.asarray(x, dtype=np.float32)
    consts = _host_smalls(
        np.asarray(Wo, np.float32), np.asarray(Uo, np.float32),
        np.asarray(Co, np.float32), np.asarray(emb_table, np.float32),
    )
    maps = []
    for c in range(N_CORES):
        xs = x[c * BS:(c + 1) * BS]                        # [BS, T, D]
        xtc = np.ascontiguousarray(xs.transpose(0, 2, 1))  # [BS, D, T]
        maps.append({"xt": xtc, "consts": consts})
    return maps


def _assemble(results):
    outs = []
    for c in range(N_CORES):
        o = np.asarray(results[c]["out"])                  # [V, BS*T]
        o = o.reshape(V, BS, T).transpose(1, 2, 0)         # [BS, T, V]
        outs.append(np.ascontiguousarray(o))
    return np.concatenate(outs, axis=0)                    # [B, T, V]


def _get_nc() -> bass.Bass:
    if "nc" not in _NC_CACHE:
        _NC_CACHE["nc"] = _build_nc()
    return _NC_CACHE["nc"]


def _run(inputs: dict, trace: bool = False):
    nc = _get_nc()
    maps = _in_maps(
        inputs["x"], inputs["Wo"], inputs["Uo"], inputs["Co"],
        inputs["emb_table"],
    )
    res = run_bass_kernel_spmd(nc, maps, list(range(N_CORES)), trace=trace)
    return res


def kernel(**inputs) -> np.ndarray:
    res = _run(inputs, trace=False)
    return _assemble(res.results)


# revision 42
# speedup vs baseline: 1.1551x; 1.1551x over previous
"""Trainium2 Bass kernel for nn_CascadedAttention (B=64, T=512, D=1024, V=28).

Math notes (why this is NOT a 512-step sequential scan on device):

  reference computes, per step t with carry y_prev (y_{-1} = 0):
    scores = softmax(tanh(...) @ Va, axis=-1)     # softmax over a SIZE-1 axis
                                                  # -> exactly 1.0 everywhere
    c      = einsum('btd,bt->bd', x, scores)      # -> x.sum(axis=1), step-invariant
    idx    = int32(y_prev)                        # y_prev in (0,1] -> idx in {0,1};
                                                  # idx==1 iff y_prev == 1.0 (fp32-saturated sigmoid)
    WoE    = emb_table[idx] @ Wo                  # -> w0 + (w1-w0)*idx elementwise
    y      = sigmoid(WoE + h_prev @ Uo + c @ Co)  # h_prev = x[:, t-1] (0 at t=0)

  So with G[b,t,v] = (x[b] @ Uo)[t,v], bias[b,v] = w0 + (c@Co)[b,v],
  delta = w1 - w0, and the binary state s_t = 1[G[t-1] + bias + delta*s_{t-1} >= theta]
  (theta = fp32 sigmoid saturation threshold; G[-1] := 0), the outputs are
      y_t = sigmoid(G[t-1] + bias + delta * s_{t-1}).
  s_t follows p0_t + (p1_t - p0_t)*s_{t-1} with p0_t = 1[G[t-1] >= theta-bias],
  p1_t = 1[G[t-1] >= theta-bias-delta], which maps exactly onto the DVE
  tensor_tensor_scan primitive (state = data0*state + data1): ONE instruction
  per batch-group. Wa, Ua, Va are mathematically dead (all-ones softmax).

Sharding: data-parallel over batch, 8 batches per core; x pre-transposed on
host to [BS, D, T] so every load is one contiguous [128, T] block.

Toolchain constraints that shaped the structure (nix walrus 2026-05):
  * ONE sync wait per instruction. Hence: warm-up consumers per engine for
    the constants, unique input tiles (no slot-recycling waits), a reserved
    DMA bookkeeping lane for the single output store (lane-first => its only
    wait is the sigmoid), and a patched Tile tail drain that splits its
    N-sem wait list into a chain of single-wait drains.
  * PE matmul psum writes only at partition bases {0, 32, 64}: two batches
    share a psum tile at bases 0/64 with stacked [Uo|pad|Co] weights.
"""

import numpy as np

import concourse.bass as bass
import concourse.mybir as mybir
import concourse.tile as _tile_mod
import concourse.tile_sem_assignment as _tsa
from concourse.tile import TileContext
from concourse.tile_scheduler import DMAInst
from concourse.vector_clock import ScopedClock
from concourse.bass_utils import run_bass_kernel_spmd

B, T, D, V = 64, 512, 1024, 28
N_CORES = 8
BS = B // N_CORES          # batches per core
KC = D // 128              # contraction chunks
NG = BS // 2               # psum pair-groups per core
F32 = mybir.dt.float32
# smallest fp32 x with 1/(1+exp(-x)) == 1.0 (24*ln2). Any value in [16, 19]
# yields indistinguishable outputs (see derivation above: a theta mismatch only
# flips idx where the NEXT sigmoid is saturated, shifting y by < 1e-6).
THETA = 16.635532333438687

CW = 64                    # packed weight chunk: 0:28 Uo, 32:60 Co, rest pad
WD = KC * CW               # column of [w0, delta, theta, theta-delta] scalars
NCONST = WD + 4

_NC_CACHE: dict = {}


# ---- Tile framework patches for the 1-wait-per-instruction walrus build ----

def _split_drain_and_barrier(self, tick_clock, wait_clock):
    """Tail drain: split its N-sem wait list into single-wait drains on SP."""
    nc = self.nc
    drain_inst = nc.sync.drain()
    wait_clock.add_sem_waits(
        drain_inst.ins, ScopedClock({None: tick_clock.global_clock})
    )
    si = drain_inst.ins.sync_info
    waits = list(si.on_wait) if si is not None and si.on_wait else []
    upds = list(si.on_update) if si is not None and si.on_update else []
    if len(waits) > 1:
        drain_inst.ins.sync_info = mybir.SyncInfo(on_wait=[waits[0]], on_update=[])
        for i, w in enumerate(waits[1:]):
            d2 = nc.sync.drain()
            last = i == len(waits) - 2
            d2.ins.sync_info = mybir.SyncInfo(
                on_wait=[w], on_update=upds if last else []
            )

    nc.all_engine_barrier()
    assert self.sems is not None
    popped = nc._tile_sem_poison_stack.pop()
    assert popped is self._sem_poison
    nc.clear_and_free_semaphores(list(self.sems.allocated().values()))
    nc.all_engine_barrier()


_tile_mod.TileContext._drain_and_barrier = _split_drain_and_barrier

# Reserve HWDGE bookkeeping lane 7 for instructions in _PIN_LANE7 (the output
# store): being lane-first, the store carries only its producer wait. All other
# HWDGE DMAs round-robin lanes 0-6.
_PIN_LANE7: set = set()
_orig_assign_tick = _tsa.TileClockTick._assign_tick


def _assign_tick_pin(self, inst):
    if isinstance(inst, DMAInst) and inst.engine != mybir.EngineType.Pool:
        if inst.name in _PIN_LANE7:
            self.next_hw_dma_idx = 7
        elif self.next_hw_dma_idx == 7:
            self.next_hw_dma_idx = 0
    return _orig_assign_tick(self, inst)


_tsa.TileClockTick._assign_tick = _assign_tick_pin


def _build_nc() -> bass.Bass:
    nc = bass.Bass()
    xt = nc.declare_dram_parameter("xt", [BS, D, T], F32, isOutput=False)
    consts = nc.declare_dram_parameter("consts", [128, NCONST], F32, isOutput=False)
    # output rows {0:28, 64:92} = batch {2g, 2g+1}, cols g*T+t; rest junk
    out = nc.declare_dram_parameter("out", [92, NG * T], F32, isOutput=True)

    with TileContext(nc) as tc:
        with (
            tc.tile_pool(name="consts_p", bufs=1) as cpool,
            tc.tile_pool(name="xin", bufs=1) as xpool,
            tc.tile_pool(name="mid", bufs=4) as mpool,
            tc.tile_pool(name="scan", bufs=2) as spool,
            tc.tile_pool(name="psum", bufs=NG, space="PSUM") as ppool,
        ):
            cb = cpool.tile([128, NCONST], F32)
            nc.sync.dma_start(out=cb[:], in_=consts[:])
            # DVE warm-up consumption so later DVE users carry no DMA wait
            junk = cpool.tile([1, 4], F32)
            nc.vector.tensor_copy(junk[:], cb[0:1, WD:WD + 4])

            # z for all 4 pair-groups side by side; zeroed so column g*T (the
            # t=0 slot) is 0 and junk rows stay finite
            z_all = cpool.tile([92, NG * T], F32)
            y_all = cpool.tile([92, NG * T], F32)
            nc.vector.memset(z_all[:], 0.0)

            ps_tiles = [
                ppool.tile([128, T], F32, tag="ps", name=f"ps{i}")
                for i in range(NG)
            ]
            # PE warm-up matmul consuming the consts DMA so no later matmul
            # needs more than one wait
            nc.tensor.matmul(
                ps_tiles[0][0:1, 0:1], cb[:, 0:1], cb[:, 0:1],
                start=True, stop=True,
            )

            # x loads: one [128, T] tile per (b, k), unique (no recycling
            # waits); 64 sequential 256 KiB direct2d transfers keep the DGE
            # ring dense at full HBM rate
            xk_tiles = {}
            for b in range(BS):
                for k in range(KC):
                    xk = xpool.tile(
                        [128, T], F32, tag=f"xk{b}_{k}", name=f"xk{b}_{k}"
                    )
                    nc.sync.dma_start(
                        out=xk[:], in_=xt[b, k * 128:(k + 1) * 128, :]
                    )
                    xk_tiles[b, k] = xk
                # one matmul per chunk: [Uo|pad|Co] stacked -> G rows at
                # base 64*(b%2), CC rows 32 above
                base = 64 * (b % 2)
                ps = ps_tiles[b // 2]
                for k in range(KC):
                    nc.tensor.matmul(
                        ps[base:base + CW, :],
                        cb[:, k * CW:(k + 1) * CW], xk_tiles[b, k][:],
                        start=(k == 0), stop=(k == KC - 1),
                    )

            for g in range(NG):
                ps = ps_tiles[g]
                zc = g * T     # this group's column block in z_all/y_all
                z0 = z_all[:, zc:zc + 1]  # always-zero column (memset)

                # bias[b] = w0 + sum_t CC.T: full-tile reduce, then shift the
                # CC rows (32:60, 96:124) down onto the G rows (0:28, 64:92)
                br = mpool.tile([124, 1], F32, tag="br")
                nc.vector.tensor_reduce(
                    out=br[:], in_=ps[0:124, :],
                    axis=mybir.AxisListType.X, op=mybir.AluOpType.add,
                )
                sb = mpool.tile([92, 1], F32, tag="sb")
                nc.vector.memset(sb[:], 0.0)
                nc.vector.tensor_copy(sb[0:28, :], br[32:60, :])
                nc.vector.tensor_copy(sb[64:92, :], br[96:124, :])
                nc.vector.tensor_scalar_add(sb[:], sb[:], cb[0:92, WD:WD + 1])
                # thresholds: tmb = theta - bias, tmbd = theta - bias - delta
                tmb = mpool.tile([92, 1], F32, tag="tmb")
                nc.vector.tensor_scalar(
                    out=tmb[:], in0=sb[:], scalar1=-1.0, scalar2=float(THETA),
                    op0=mybir.AluOpType.mult, op1=mybir.AluOpType.add,
                )
                tmbd = mpool.tile([92, 1], F32, tag="tmbd")
                nc.vector.tensor_scalar_sub(tmbd[:], tmb[:], cb[0:92, WD + 1:WD + 2])

                # p0/p1 indicators straight from psum (G rows; mid rows junk)
                p0 = spool.tile([92, T], F32, tag="p0")
                d01 = spool.tile([92, T], F32, tag="d01")
                bt = spool.tile([92, T], F32, tag="bt")
                nc.vector.tensor_scalar(
                    out=p0[:, 1:T], in0=ps[0:92, 0:T - 1], scalar1=tmb[:],
                    scalar2=None, op0=mybir.AluOpType.is_ge,
                )
                nc.vector.tensor_scalar(
                    out=p0[:, 0:1], in0=z0, scalar1=tmb[:],
                    scalar2=None, op0=mybir.AluOpType.is_ge,
                )
                nc.vector.tensor_scalar(
                    out=d01[:, 1:T], in0=ps[0:92, 0:T - 1], scalar1=tmbd[:],
                    scalar2=None, op0=mybir.AluOpType.is_ge,
                )
                nc.vector.tensor_copy(d01[:, 0:1], z0)  # any finite value
                nc.vector.tensor_sub(d01[:], d01[:], p0[:])
                # s_t = d01_t * s_{t-1} + p0_t   (exact on {0,1})
                nc.vector.tensor_tensor_scan(
                    out=bt[:], data0=d01[:], data1=p0[:], initial=0.0,
                    op0=mybir.AluOpType.mult, op1=mybir.AluOpType.add,
                )
                # z_t = G[t-1] + delta * s_{t-1}  (bias added by the sigmoid)
                nc.vector.scalar_tensor_tensor(
                    out=z_all[:, zc + 1:zc + T], in0=bt[:, 0:T - 1],
                    scalar=cb[0:92, WD + 1:WD + 2], in1=ps[0:92, 0:T - 1],
                    op0=mybir.AluOpType.mult, op1=mybir.AluOpType.add,
                )
                # y = sigmoid(z + bias)
                nc.scalar.activation(
                    out=y_all[:, zc:zc + T], in_=z_all[:, zc:zc + T],
                    func=mybir.ActivationFunctionType.Sigmoid,
                    bias=sb[:], scale=1.0,
                )

            st = nc.sync.dma_start(out=out[:], in_=y_all[:])
            _PIN_LANE7.add(st.ins.name)

    return nc


def _host_smalls(Wo, Uo, Co, emb_table):
    w0 = np.float32(emb_table[0].astype(np.float32) @ Wo[:, 0].astype(np.float32))
    w1 = np.float32(emb_table[1].astype(np.float32) @ Wo[:, 0].astype(np.float32))
    delta = np.float32(w1 - w0)
    theta = np.float32(THETA)
    uoco = np.zeros((D, CW), np.float32)
    uoco[:, 0:V] = Uo
    uoco[:, 32:32 + V] = Co
    consts = np.zeros((128, NCONST), np.float32)
    consts[:, 0:WD] = (
        uoco.reshape(KC, 128, CW).transpose(1, 0, 2).reshape(128, WD)
    )
    consts[:, WD:] = np.array(
        [w0, delta, theta, np.float32(theta - delta)], np.float32
    )
    return np.ascontiguousarray(consts)


def _in_maps(x, Wo, Uo, Co, emb_table):
    x = np.asarray(x, dtype=np.float32)
    consts = _host_smalls(
        np.asarray(Wo, np.float32), np.asarray(Uo, np.float32),
        np.asarray(Co, np.float32), np.asarray(emb_table, np.float32),
    )
    maps = []
    for c in range(N_CORES):
        xs = x[c * BS:(c + 1) * BS]                        # [BS, T, D]
        xtc = np.ascontiguousarray(xs.transpose(0, 2, 1))  # [BS, D, T]
        maps.append({"xt": xtc, "consts": consts})
    return maps


def _assemble(results):
    outs = []
    for c in range(len(results)):
        o = np.asarray(results[c]["out"]).reshape(92, NG, T)
        core = np.empty((BS, T, V), np.float32)
        core[0::2] = o[0:28].transpose(1, 2, 0)            # rows 0:28  = even b
        core[1::2] = o[64:92].transpose(1, 2, 0)           # rows 64:92 = odd b
        outs.append(core)
    return np.concatenate(outs, axis=0)                    # [B, T, V]


def _get_nc() -> bass.Bass:
    if "nc" not in _NC_CACHE:
        _NC_CACHE["nc"] = _build_nc()
    return _NC_CACHE["nc"]


def _run(inputs: dict, trace: bool = False):
    nc = _get_nc()
    maps = _in_maps(
        inputs["x"], inputs["Wo"], inputs["Uo"], inputs["Co"],
        inputs["emb_table"],
    )
    res = run_bass_kernel_spmd(nc, maps, list(range(N_CORES)), trace=trace)
    return res


def kernel(**inputs) -> np.ndarray:
    res = _run(inputs, trace=False)
    return _assemble(res.results)
